# revision 8
# baseline (speedup 1.0000x reference)
"""Cosine-similarity loss on Trainium2 — 8-core SPMD Bass/Tile kernel.

Math (per token, logits row l of length V, target t):
    probs = softmax(l);  cos = probs[t] / ||probs||_2
  The softmax normalizer cancels in the ratio:
    cos = exp(l_t) / sqrt(sum_i exp(2*l_i))
  (no max-subtraction needed: logits are N(0,1) so exp(2*l) stays far below
  fp32 overflow, and ||probs|| >= 1/sqrt(V) >> eps so the eps clamps in the
  reference never fire).
  loss = 1 - sum(cos * mask) / (sum(mask) + 1e-8),  mask = (t != 0)

Sharding: tokens (B*S = 4096) are split evenly across 8 NeuronCores, 512
tokens per core.  Each core lays its 512 tokens out as 4 tiles of 128
partitions and streams the vocab axis in 4 chunks of 8000 fp32.  A single
ScalarE Exp instruction per chunk (scale=2.0, accum_out) produces the
per-token sum of exp(2*l) with no VectorE pass over the bulk data, so the
kernel is purely DMA-bound (~65.5 MB/core at ~360 GB/s).  Target logits are
gathered with an indirect DMA.  Each core returns per-partition partial sums
of cos*mask and mask; the host adds 8x128 partials and finishes the division.
"""

import numpy as np

import concourse.bacc as bacc
import concourse.bass as bass
import concourse.mybir as mybir
import concourse.tile as tile
from concourse.bass_utils import run_bass_kernel_spmd

B, S, V = 2, 2048, 32000
N_CORES = 8
NTOK = B * S                      # 4096
TOK_PER_CORE = NTOK // N_CORES    # 512
P = 128
TILES = TOK_PER_CORE // P         # 4 token tiles per core
CHUNK = 8000
NCHUNK = V // CHUNK               # 4 vocab chunks
EPS_MEAN = 1e-8


def build_program(tok_per_core=TOK_PER_CORE, v=V, chunk=CHUNK, bufs=4):
    """Build + compile the per-core Bass program (identical on all cores)."""
    tiles = tok_per_core // P
    nchunk = v // chunk
    assert tiles * P == tok_per_core and nchunk * chunk == v

    # NOTE: no num_devices — the per-core programs are fully independent
    # (no collectives; the host combines per-core partials), and num_devices>1
    # makes Tile emit a cross-device exit barrier that crashes under the axon
    # PJRT shim.
    nc = bacc.Bacc("TRN2", target_bir_lowering=False, debug=False)
    f32 = mybir.dt.float32
    i32 = mybir.dt.int32
    AF = mybir.ActivationFunctionType
    ALU = mybir.AluOpType
    AX = mybir.AxisListType

    logits = nc.dram_tensor("logits", [tok_per_core, v], f32, kind="ExternalInput").ap()
    gidx = nc.dram_tensor("gidx", [P, tiles], i32, kind="ExternalInput").ap()
    maskf = nc.dram_tensor("maskf", [P, tiles], f32, kind="ExternalInput").ap()
    out = nc.dram_tensor("out", [P, 2], f32, kind="ExternalOutput").ap()

    # Element-gather view for the indirect DMA: [tok*v, 1] (DMA APs must be 2-D)
    logits_flat = logits.rearrange("a b -> (a b)").rearrange("(a b) -> a b", b=1)

    with tile.TileContext(nc) as tc:
        with (
            tc.tile_pool(name="data", bufs=bufs) as data,
            tc.tile_pool(name="small", bufs=1) as small,
        ):
            # Main streaming pass FIRST in program order so the ACT engine's
            # chunk Exps start as soon as chunk 0 lands (the gathers below take
            # ~15us of SWDGE time and must not gate the ACT stream).
            # s2acc[p, t*nchunk+c] = sum_j exp(2*chunk[p, j])
            s2acc = small.tile([P, tiles * nchunk], f32)
            for t in range(tiles):
                for c in range(nchunk):
                    ch = data.tile([P, chunk], f32, tag="chunk")
                    nc.sync.dma_start(
                        out=ch[:],
                        in_=logits[t * P : (t + 1) * P, c * chunk : (c + 1) * chunk],
                    )
                    col = t * nchunk + c
                    nc.scalar.activation(
                        out=ch[:],
                        in_=ch[:],
                        func=AF.Exp,
                        scale=2.0,
                        accum_out=s2acc[:, col : col + 1],
                    )

            gidx_sb = small.tile([P, tiles], i32)
            mask_sb = small.tile([P, tiles], f32)
            nc.sync.dma_start(out=gidx_sb[:], in_=gidx)
            nc.sync.dma_start(out=mask_sb[:], in_=maskf)

            # Gather the target logit of each token: lt[p, t] = logits.flat[gidx[p, t]]
            lt = small.tile([P, tiles], f32)
            for t in range(tiles):
                nc.gpsimd.indirect_dma_start(
                    out=lt[:, t : t + 1],
                    out_offset=None,
                    in_=logits_flat,
                    in_offset=bass.IndirectOffsetOnAxis(
                        ap=gidx_sb[:, t : t + 1], axis=0
                    ),
                )
            exp_lt = small.tile([P, tiles], f32)
            nc.scalar.activation(out=exp_lt[:], in_=lt[:], func=AF.Exp)

            # s2[p, t] = sum_c s2acc[p, t, c]
            s2 = small.tile([P, tiles], f32)
            nc.vector.tensor_reduce(
                out=s2[:],
                in_=s2acc[:].rearrange("p (t c) -> p t c", c=nchunk),
                axis=AX.X,
                op=ALU.add,
            )
            # rs = 1/sqrt(s2): exact DVE reciprocal, then ACT sqrt
            recip = small.tile([P, tiles], f32)
            nc.vector.reciprocal(out=recip[:], in_=s2[:])
            rs = small.tile([P, tiles], f32)
            nc.scalar.activation(out=rs[:], in_=recip[:], func=AF.Sqrt)

            cosv = small.tile([P, tiles], f32)
            nc.vector.tensor_mul(cosv[:], exp_lt[:], rs[:])
            cosm = small.tile([P, tiles], f32)
            nc.vector.tensor_mul(cosm[:], cosv[:], mask_sb[:])

            # res[:, 0] = sum_t cos*mask ; res[:, 1] = sum_t mask
            res = small.tile([P, 2], f32)
            nc.vector.tensor_reduce(
                out=res[:, 0:1], in_=cosm[:], axis=AX.X, op=ALU.add
            )
            nc.vector.tensor_reduce(
                out=res[:, 1:2], in_=mask_sb[:], axis=AX.X, op=ALU.add
            )
            nc.sync.dma_start(out=out, in_=res[:])

    nc.compile()
    return nc


_NC_CACHE = {}


def _get_nc():
    if "nc" not in _NC_CACHE:
        _NC_CACHE["nc"] = build_program()
    return _NC_CACHE["nc"]


def make_in_maps(logits, targets):
    """Shard full inputs into per-core input maps (host-side prep only)."""
    logits = np.asarray(logits)
    targets = np.asarray(targets)
    assert logits.shape == (B, S, V), logits.shape
    lf = np.ascontiguousarray(logits.reshape(NTOK, V).astype(np.float32, copy=False))
    tf = targets.reshape(NTOK).astype(np.int64)

    # token j of a core sits at (partition p = j % P, tile t = j // P)
    local_tok = (np.arange(TILES)[None, :] * P + np.arange(P)[:, None]).astype(np.int64)

    in_maps = []
    for k in range(N_CORES):
        sl = slice(k * TOK_PER_CORE, (k + 1) * TOK_PER_CORE)
        tk = tf[sl].reshape(TILES, P).T          # [P, TILES]
        gidx = (local_tok * V + tk).astype(np.int32)
        in_maps.append(
            {
                "logits": lf[sl],
                "gidx": np.ascontiguousarray(gidx),
                "maskf": np.ascontiguousarray((tk != 0).astype(np.float32)),
            }
        )
    return in_maps


def reduce_outputs(per_core_outs):
    """Combine per-core [128, 2] partials into the final scalar loss."""
    s = 0.0
    c = 0.0
    for o in per_core_outs:
        s += float(o[:, 0].astype(np.float64).sum())
        c += float(o[:, 1].astype(np.float64).sum())
    return np.asarray(np.float32(1.0 - s / (c + EPS_MEAN)))


def run_on_device(in_maps, **kwargs):
    nc = _get_nc()
    return run_bass_kernel_spmd(nc, in_maps, core_ids=list(range(N_CORES)), **kwargs)


def kernel(logits, targets):
    in_maps = make_in_maps(logits, targets)
    res = run_on_device(in_maps)
    return reduce_outputs([r["out"] for r in res.results])
